# revision 17
# baseline (speedup 1.0000x reference)
"""CP-decomposed conv (1x1 -> depthwise-h -> depthwise-w -> 1x1) on 8 TRN2
NeuronCores, data-parallel over batch (4 images per core).

Per-core pipeline:
  stage A: u[r, h', w] = sum_{a,c} x[c, h'+a, w] * W1[(a,c), r]
           (h-depthwise folded into the channel-mixing matmul via
            host-precomputed W1 = f1 (x) f3; bf16 matmuls, fp32 PSUM accum)
  w-taps:  z[r, h', w'] = sum_cw u[r, h', w'+cw] * f2[cw, r], computed as
             ze = u * f2[0]            (fused into PSUM->SBUF copy, ACT)
             z  = ze[+1]*r1 + ze[+0]   (DVE)    r_c = f2[cw]/f2[0]
             z  = ze[+2]*r2 + z        (DVE)
  stage B: out[f, h', w'] = sum_r f0[f, r] * z[r, h', w']  (bf16 matmul)
Output is stored bf16 in a DMA-friendly [img, ftile, p, h'*w'] layout and
reshaped/upcast on host.
"""

import numpy as np

B, C, H, W = 32, 256, 128, 128
FH, FW = 3, 3
F, R = 256, 128
HP, WP = H - FH + 1, W - FW + 1  # 126, 126
NCORES = 8
BL = B // NCORES  # images per core

# 126 output rows: short first band (fast pipeline ramp), 15x8, short tail.
BANDS = [(0, 4)] + [(4 + i * 12, 12) for i in range(10)] + [(124, 2)]


def _chunks(bh):
    out = []
    r0 = 0
    while r0 < bh:
        nr = min(4, bh - r0)
        out.append((r0, nr))
        r0 += nr
    return out


_NC_CACHE = {}


def _build_nc():
    import concourse.bacc as bacc
    import concourse.mybir as mybir
    import concourse.tile as tile

    dt = mybir.dt
    bf16 = dt.bfloat16
    f32 = dt.float32
    f32r = dt.float32r
    mult = mybir.AluOpType.mult
    add = mybir.AluOpType.add

    nc = bacc.Bacc("TRN2", target_bir_lowering=False, debug=False,
                   num_devices=NCORES)

    x_d = nc.dram_tensor("x", [BL, C, H, W], bf16, kind="ExternalInput").ap()
    w1_d = nc.dram_tensor("w1", [FH * C, R], bf16, kind="ExternalInput").ap()
    f0t_d = nc.dram_tensor("f0t", [R, F], bf16, kind="ExternalInput").ap()
    # f2s[r] = [f2[0,r], f2[1,r]/f2[0,r], f2[2,r]/f2[0,r]]
    f2s_d = nc.dram_tensor("f2s", [R, FW], f32, kind="ExternalInput").ap()
    out_d = nc.dram_tensor("out", [BL, 2, 128, HP * WP], bf16,
                           kind="ExternalOutput").ap()

    with tile.TileContext(nc, trace_sim=False) as tc:
        with tc.tile_pool(name="wp", bufs=1) as wp, \
             tc.tile_pool(name="xp", bufs=2) as xp, \
             tc.tile_pool(name="ep", bufs=4) as ep, \
             tc.tile_pool(name="zp", bufs=4) as zp, \
             tc.tile_pool(name="op", bufs=4) as op, \
             tc.tile_pool(name="ups", bufs=2, space="PSUM") as upsp, \
             tc.tile_pool(name="ops", bufs=2, space="PSUM") as opsp:

            # --- weights (resident) ---
            w1_t = wp.tile([128, FH * 2, 128], bf16)  # [c_sub, kt=a*2+ct, r]
            nc.scalar.dma_start(
                w1_t[:, :, :],
                w1_d.rearrange("(kt p) r -> p kt r", p=128),
            )
            f2s_t = wp.tile([128, FW], f32)
            nc.scalar.dma_start(f2s_t[:, :], f2s_d)
            f0t_t = wp.tile([128, F], bf16)
            nc.scalar.dma_start(f0t_t[:, :], f0t_d)

            ci = 0
            for img in range(BL):
                # --- load x image: [c_sub, ct, h*w] ---
                x_t = xp.tile([128, 2, H * W], bf16, tag="x")
                qparts = ([(0, 8), (8, 8), (16, 16)] if img == 0
                          else [(0, 32)]) \
                    + [(32 * q, 32) for q in range(1, 4)]
                for (row0, nrow) in qparts:
                    for ct in range(2):
                        nc.sync.dma_start(
                            x_t[:, ct, row0 * 128:(row0 + nrow) * 128],
                            x_d[img, ct * 128:(ct + 1) * 128,
                                row0:row0 + nrow, :],
                        )

                for (h0, bh) in BANDS:
                    # --- stage A: u in PSUM [r, bh*128]; weight-outer so
                    # the stationary tile stays put across chunks ---
                    u_ps = upsp.tile([128, 12 * 128], f32, tag="u")
                    for a in range(FH):
                        for ct in range(2):
                            for (r0, nr) in _chunks(bh):
                                n = nr * 128
                                row = h0 + r0 + a
                                nc.tensor.matmul(
                                    u_ps[:, r0 * 128: r0 * 128 + n],
                                    w1_t[:, a * 2 + ct, :],
                                    x_t[:, ct, row * 128: row * 128 + n],
                                    start=(a == 0 and ct == 0),
                                    stop=(a == FH - 1 and ct == 1),
                                )

                    # --- fused PSUM->SBUF copy * f2[0] (tap0), then taps ---
                    ze_t = ep.tile([128, 12 * 128], bf16, tag="ze")
                    nc.scalar.mul(ze_t[:, 0:bh * 128], u_ps[:, 0:bh * 128],
                                  f2s_t[:, 0:1])
                    z_t = zp.tile([128, 12 * WP], bf16, tag="z")
                    zv = z_t[:, 0:bh * WP].rearrange("p (h w) -> p h w", w=WP)
                    zev = ze_t[:, 0:bh * 128].rearrange(
                        "p (h w) -> p h w", w=128)
                    nc.vector.scalar_tensor_tensor(
                        zv, zev[:, :, 1:1 + WP], f2s_t[:, 1:2],
                        zev[:, :, 0:WP], op0=mult, op1=add)
                    nc.vector.scalar_tensor_tensor(
                        zv, zev[:, :, 2:2 + WP], f2s_t[:, 2:3],
                        zv, op0=mult, op1=add)

                    # --- stage B (bf16) + PSUM->SBUF copy ---
                    o_t = op.tile([128, 2, 12 * WP], bf16, tag="o")
                    for ft in range(2):
                        for (r0, nr) in _chunks(bh):
                            n = nr * WP
                            o_ps = opsp.tile([128, 504], f32, tag="ops")
                            nc.tensor.matmul(
                                o_ps[:, 0:n],
                                f0t_t[:, ft * 128:(ft + 1) * 128],
                                z_t[:, r0 * WP: r0 * WP + n],
                                start=True, stop=True,
                            )
                            dst = o_t[:, ft, r0 * WP: r0 * WP + n]
                            if ci % 4 == 3:
                                nc.vector.tensor_copy(dst, o_ps[:, 0:n])
                            else:
                                nc.scalar.copy(dst, o_ps[:, 0:n])
                            ci += 1

                    # --- store band (contiguous per partition) ---
                    nc.sync.dma_start(
                        out_d[img, :, :, h0 * WP:(h0 + bh) * WP].rearrange(
                            "f p n -> p f n"),
                        o_t[:, :, 0: bh * WP],
                    )

    nc.compile()
    return nc


def _get_nc():
    if "nc" not in _NC_CACHE:
        _NC_CACHE["nc"] = _build_nc()
    return _NC_CACHE["nc"]


def _prep_in_maps(x, f0, f1, f2, f3):
    import ml_dtypes
    bf16 = ml_dtypes.bfloat16

    # W1[(a, c), r] = f1[a, r] * f3[c, r]
    w1 = (np.asarray(f1, np.float32)[:, None, :]
          * np.asarray(f3, np.float32)[None, :, :]).reshape(FH * C, R)
    w1b = np.ascontiguousarray(w1.astype(bf16))
    f0t = np.ascontiguousarray(np.asarray(f0, np.float32).T.astype(bf16))
    f2 = np.asarray(f2, np.float64)
    s0 = f2[0].copy()
    s0[np.abs(s0) < 1e-30] = 1e-30
    f2s = np.stack([s0, f2[1] / s0, f2[2] / s0], axis=1).astype(np.float32)
    f2s = np.ascontiguousarray(f2s)
    xb = np.ascontiguousarray(np.asarray(x).astype(bf16))
    return [
        {"x": xb[i * BL:(i + 1) * BL], "w1": w1b, "f0t": f0t, "f2s": f2s}
        for i in range(NCORES)
    ]


def kernel(x, f0, f1, f2, f3):
    from concourse import bass_utils

    nc = _get_nc()
    in_maps = _prep_in_maps(x, f0, f1, f2, f3)
    res = bass_utils.run_bass_kernel_spmd(
        nc, in_maps, core_ids=list(range(NCORES)))
    # out shards are [BL, 2, 128, HP*WP]; (ft, p) merges to F contiguously.
    # bf16 -> fp32 via bit shift (exact, much faster than ml_dtypes astype).
    shards = [np.asarray(r["out"]).view(np.uint16) for r in res.results]
    raw = np.stack(shards)  # [NCORES, BL, 2, 128, HP*WP] uint16
    out = (raw.astype(np.uint32) << 16).view(np.float32)
    return np.ascontiguousarray(out.reshape(B, F, HP, WP))
